# revision 9
# baseline (speedup 1.0000x reference)
"""NSFFM Trainium2 kernel: 8-way sequence-parallel, q-space segmented scan.

Self-contained: builds a Bass/Tile program once (cached), runs it SPMD on 8
NeuronCores via run_bass_kernel_spmd, gathers per-core outputs on the host.

Algorithm per layer (validated against the jax reference in fp32):
  tr = x@Wtr.T, ct = x@Wc.T ; p[t,i,cc] = |tr_i||ct_cc| / (1e-8 + sum)
  complex state s_t = (1-start_t) * lam * s_{t-1} + p_t,  lam = e^{-|a_i|} e^{i b_cc}
  derotated q_t = e^{-i b t} s_t obeys  q_t = d_i m_t q_{t-1} + p_t e^{-i b t}
  -> two real tensor_tensor_scans per 128-channel chunk (channels = partitions,
     time = free dim).  Cross-core carry: per-core monoid (B_c, noreset flag)
  AllGathered; each core combines prefixes with a host-fed mask and applies the
  correction q += P[t] d^{t+1} q_init as one fused scalar_tensor_tensor.
  features: |s|=|q|, mag=log1p|q|, G=mag/|q| (ln/exp), then phase rotation by
  e^{i b t} fused with G; two matmuls + layernorm (exact leaky-relu via max).
"""

import numpy as np

T, D, TR, CC, L = 4096, 512, 64, 64, 3
NC_ = 8
R = T // NC_          # 512 rows per core
NCH = (TR * CC) // 128  # 32 channel chunks (i-major, 2 i-rows x 64 cc each)
TWO_PI = float(2.0 * np.pi)

_CACHE = {}


def _split_multiwaits(nc):
    """This walrus build accepts at most ONE sync-wait per instruction; hoist
    extra waits onto single-wait NoOps emitted just before on the same engine."""
    import concourse.mybir as mybir
    n_fixed = 0
    for f in nc.m.functions:
        for blk in f.blocks:
            new_list = []
            for ins in blk.instructions:
                si = getattr(ins, "sync_info", None)
                if si is not None and si.on_wait and len(si.on_wait) > 1:
                    waits = list(si.on_wait)
                    for k, w in enumerate(waits[:-1]):
                        new_list.append(mybir.InstNoOp(
                            name=f"{ins.name}-sw{k}", engine=ins.engine,
                            sync_info=mybir.SyncInfo(on_wait=[w], on_update=[]),
                            bass_nofuse=True))
                    ins.sync_info.on_wait = [waits[-1]]
                    n_fixed += 1
                new_list.append(ins)
            blk.instructions[:] = new_list
    return n_fixed


def _build():
    import concourse.bass as bass
    import concourse.tile as tile
    from concourse import mybir

    f32, f32r, i32 = mybir.dt.float32, mybir.dt.float32r, mybir.dt.int32
    MULT, ADD = mybir.AluOpType.mult, mybir.AluOpType.add
    GT, LT = mybir.AluOpType.is_gt, mybir.AluOpType.is_lt
    ACT = mybir.ActivationFunctionType

    nc = bass.Bass()

    xT = nc.declare_dram_parameter("xT", [D, R], f32r, isOutput=False)
    startf = nc.declare_dram_parameter("startf", [1, R], f32, isOutput=False)
    pmask = nc.declare_dram_parameter("pmask", [1, NC_], f32, isOutput=False)
    WtrT = nc.declare_dram_parameter("WtrT", [L, D, TR], f32r, isOutput=False)
    WcT = nc.declare_dram_parameter("WcT", [L, D, CC], f32r, isOutput=False)
    W0T = nc.declare_dram_parameter("W0T", [L, 2 * TR * CC, D], f32r, isOutput=False)
    W1T = nc.declare_dram_parameter("W1T", [L, D, D], f32r, isOutput=False)
    avec = nc.declare_dram_parameter("avec", [L, TR, 1], f32, isOutput=False)
    bvec = nc.declare_dram_parameter("bvec", [L, CC, 1], f32, isOutput=False)
    st0m = nc.declare_dram_parameter("st0m", [L, 128, NCH], f32, isOutput=False)
    lnp = nc.declare_dram_parameter("lnp", [L, 128, 24], f32, isOutput=False)

    zT = nc.declare_dram_parameter("zT", [D, R], f32, isOutput=True)

    with tile.TileContext(nc) as tc:
        import contextlib
        ctx = contextlib.ExitStack()
        with ctx:
            persist = ctx.enter_context(tc.tile_pool(name="persist", bufs=1))
            lay = ctx.enter_context(tc.tile_pool(name="lay", bufs=1))
            mats = ctx.enter_context(tc.tile_pool(name="mats", bufs=1))
            work = ctx.enter_context(tc.tile_pool(name="work", bufs=2))
            wsm = ctx.enter_context(tc.tile_pool(name="wsm", bufs=1))
            wstream = ctx.enter_context(tc.tile_pool(name="wstream", bufs=2))
            psum = ctx.enter_context(tc.tile_pool(name="psum", bufs=1, space="PSUM"))
            psum2 = ctx.enter_context(tc.tile_pool(name="psum2", bufs=2, space="PSUM"))
            dram = ctx.enter_context(tc.tile_pool(name="dram", bufs=1, space="DRAM"))

            def wt(tag, shape=None, dtype=None):
                return work.tile(shape or [128, R], dtype or f32, tag=tag,
                                 name=f"w_{tag}")

            def pb_load(dst_ap, dram_t, row, n_rows, width):
                src = bass.AP(tensor=dram_t.tensor, offset=row * width,
                              ap=[[0, n_rows], [1, width]])
                nc.sync.dma_start(dst_ap, src)

            # ---------- kernel-wide constants ----------
            ioti = wt("gA", [128, R], i32)
            nc.gpsimd.iota(ioti[:], pattern=[[1, R]], base=0, channel_multiplier=0)
            iotf = persist.tile([128, R], f32, tag="iotf")
            nc.vector.tensor_copy(iotf[:], ioti[:])

            st_sb = wt("gB", [1, R])
            nc.sync.dma_start(st_sb[:], startf[:])
            mrow = wt("gC", [1, R])
            nc.vector.tensor_scalar(mrow[:], st_sb[:], -1.0, 1.0, MULT, ADD)
            zrow = wt("gD", [1, R])
            nc.vector.memset(zrow[:], 0.0)
            prow = wt("gE", [1, R])
            nc.vector.tensor_tensor_scan(prow[:], mrow[:], zrow[:], 1.0, MULT, ADD)
            flagv = persist.tile([1, 1], f32, tag="flagv")
            nc.vector.tensor_copy(flagv[:], prow[:, R - 1:R])

            mp_d = dram.tile([2, R], f32, tag="mp_d")
            nc.sync.dma_start(mp_d[0:1, :], mrow[:])
            nc.sync.dma_start(mp_d[1:2, :], prow[:])
            M2 = persist.tile([128, R], f32, tag="M2")
            P2 = persist.tile([128, R], f32, tag="P2")
            pb_load(M2[:], mp_d, 0, 128, R)
            pb_load(P2[:], mp_d, 1, 128, R)

            pm_sb = persist.tile([1, NC_], f32, tag="pm_sb")
            nc.sync.dma_start(pm_sb[:], pmask[:])
            pm_d = dram.tile([1, NC_], f32, tag="pm_d")
            nc.sync.dma_start(pm_d[:], pm_sb[:])
            PM128 = persist.tile([128, NC_], f32, tag="PM128")
            pb_load(PM128[:], pm_d, 0, 128, NC_)
            OM128 = persist.tile([128, NC_], f32, tag="OM128")
            nc.vector.tensor_scalar(OM128[:], PM128[:], -1.0, 1.0, MULT, ADD)

            ones_f = persist.tile([128, 1], f32, tag="ones_f")
            nc.vector.memset(ones_f[:], 1.0)
            ones128 = persist.tile([128, 1], f32r, tag="ones128")
            nc.vector.tensor_copy(ones128[:], ones_f[:])
            ones64 = ones128[0:TR, :]
            c_sqrt_eps = persist.tile([128, 1], f32, tag="c_sqrt_eps")
            nc.vector.memset(c_sqrt_eps[:], 1e-20)
            c_ln_eps = persist.tile([128, 1], f32, tag="c_ln_eps")
            nc.vector.memset(c_ln_eps[:], 1e-5)

            xt_sb = [persist.tile([128, R], f32r, tag=f"xt{k}", name=f"xt{k}")
                     for k in range(4)]
            for k in range(4):
                nc.sync.dma_start(xt_sb[k][:], xT[k * 128:(k + 1) * 128, :])

            q_tiles = []
            for g in range(NCH):
                q_tiles.append(
                    (persist.tile([128, R], f32, tag=f"qr{g}", name=f"qr{g}"),
                     persist.tile([128, R], f32, tag=f"qi{g}", name=f"qi{g}")))

            def mod_sin_cos(out_s, out_c, arg, shp):
                """sin/cos(arg) for arg >= 0 of any magnitude (mod-2pi safe
                for either cast rounding mode)."""
                r = wt("gA", shp)
                nc.vector.tensor_scalar_mul(r[:], arg, 1.0 / TWO_PI)
                ri = wt("gB", shp, i32)
                nc.vector.tensor_copy(ri[:], r[:])
                rf = wt("gC", shp)
                nc.vector.tensor_copy(rf[:], ri[:])
                a2 = wt("gA", shp)
                nc.vector.scalar_tensor_tensor(a2[:], rf[:], -TWO_PI, arg, MULT, ADD)
                hi = wt("gB", shp)
                nc.vector.tensor_scalar(hi[:], a2[:], float(np.pi), 0.0, GT, ADD)
                a3 = wt("gC", shp)
                nc.vector.scalar_tensor_tensor(a3[:], hi[:], -TWO_PI, a2[:], MULT, ADD)
                lo = wt("gA", shp)
                nc.vector.tensor_scalar(lo[:], a3[:], float(-np.pi), 0.0, LT, ADD)
                a4 = wt("gB", shp)
                nc.vector.scalar_tensor_tensor(a4[:], lo[:], TWO_PI, a3[:], MULT, ADD)
                nc.scalar.activation(out_s, a4[:], ACT.Sin)
                c1 = wt("gC", shp)
                nc.vector.tensor_scalar_add(c1[:], a4[:], float(np.pi / 2))
                hi2 = wt("gA", shp)
                nc.vector.tensor_scalar(hi2[:], c1[:], float(np.pi), 0.0, GT, ADD)
                c2 = wt("gB", shp)
                nc.vector.scalar_tensor_tensor(c2[:], hi2[:], -TWO_PI, c1[:], MULT, ADD)
                nc.scalar.activation(out_c, c2[:], ACT.Sin)

            def layer_norm(yP, lnc, bias_col0, g_col0, beta_col0, zlist):
                ysb = []
                for dt in range(4):
                    ysb_t = mats.tile([128, R], f32r, tag=f"ysb{dt}",
                                      name=f"ysb{dt}")
                    nc.vector.tensor_scalar(ysb_t[:], yP[dt][:],
                                            lnc[:, bias_col0 + dt:bias_col0 + dt + 1],
                                            None, ADD)
                    ysb.append(ysb_t)
                s1P_ = psum2.tile([1, R], f32, tag="statP", name="s1P_")
                for dt in range(4):
                    nc.tensor.matmul(s1P_[:], ones128[:], ysb[dt][:],
                                     start=(dt == 0), stop=(dt == 3))
                s2P_ = psum2.tile([1, R], f32, tag="statP", name="s2P_")
                for dt in range(4):
                    ysq_t = wt("gA", None, f32r)
                    nc.scalar.activation(ysq_t[:], ysb[dt][:].bitcast(f32), ACT.Square)
                    nc.tensor.matmul(s2P_[:], ones128[:], ysq_t[:],
                                     start=(dt == 0), stop=(dt == 3))
                mu = wt("gE", [1, R])
                nc.vector.tensor_scalar_mul(mu[:], s1P_[:], 1.0 / D)
                e2 = wt("gF", [1, R])
                nc.vector.tensor_scalar_mul(e2[:], s2P_[:], 1.0 / D)
                musq = wt("gE", [1, R])
                nc.vector.tensor_mul(musq[:], mu[:], mu[:])
                var = wt("gF", [1, R])
                nc.vector.tensor_sub(var[:], e2[:], musq[:])
                mr_d = dram.tile([2, R], f32, tag="mr_d")
                nc.sync.dma_start(mr_d[0:1, :], mu[:])
                sd = wt("gE", [1, R])
                nc.scalar.activation(sd[:], var[:], ACT.Sqrt, bias=c_ln_eps[0:1, :])
                rstd = wt("gF", [1, R])
                nc.vector.reciprocal(rstd[:], sd[:])
                nc.sync.dma_start(mr_d[1:2, :], rstd[:])
                MU128 = wt("gB")
                RS128 = wt("gC")
                pb_load(MU128[:], mr_d, 0, 128, R)
                pb_load(RS128[:], mr_d, 1, 128, R)
                outs = []
                for dt in range(4):
                    u = wt("gD")
                    nc.vector.tensor_sub(u[:], ysb[dt][:].bitcast(f32), MU128[:])
                    v = wt("gE")
                    nc.vector.tensor_mul(v[:], u[:], RS128[:])
                    w = wt("gF")
                    nc.vector.scalar_tensor_tensor(
                        w[:], v[:], lnc[:, g_col0 + dt:g_col0 + dt + 1],
                        lnc[:, beta_col0 + dt:beta_col0 + dt + 1]
                        .broadcast_to([128, R]), MULT, ADD)
                    wz = wt("gD")
                    nc.vector.tensor_scalar_mul(wz[:], w[:], 0.01)
                    z = mats.tile([128, R], f32r, tag=f"z{dt}", name=f"z{dt}")
                    nc.vector.tensor_max(z[:], w[:], wz[:])
                    outs.append(z)
                return outs

            for l in range(L):
                # ---------- per-layer tables ----------
                acol = wsm.tile([TR, 1], f32, tag="acol", name="acol")
                nc.sync.dma_start(acol[:], avec[l])
                bcol = wsm.tile([CC, 1], f32, tag="bcol", name="bcol")
                nc.sync.dma_start(bcol[:], bvec[l])
                amc = wsm.tile([TR, 1], f32, tag="amc", name="amc")
                nc.scalar.activation(amc[:], acol[:], ACT.Abs)
                negam = wsm.tile([TR, 1], f32, tag="negam", name="negam")
                nc.vector.tensor_scalar_mul(negam[:], amc[:], -1.0)
                dcol = wsm.tile([TR, 1], f32, tag="dcol", name="dcol")
                nc.scalar.activation(dcol[:], amc[:], ACT.Exp, scale=-1.0)
                d512c = wsm.tile([TR, 1], f32, tag="d512c", name="d512c")
                nc.scalar.activation(d512c[:], amc[:], ACT.Exp, scale=-float(R))

                sc_d = dram.tile([1, 128], f32, tag="sc_d")
                nc.sync.dma_start(
                    bass.AP(tensor=sc_d.tensor, offset=0, ap=[[1, CC], [1, 1]]),
                    bcol[:])
                b2col = wsm.tile([128, 1], f32, tag="b2col", name="b2col")
                for half in range(2):
                    nc.sync.dma_start(
                        b2col[64 * half:64 * half + 64, :],
                        bass.AP(tensor=sc_d.tensor, offset=0, ap=[[1, CC], [1, 1]]))

                argbt = wt("gF")
                nc.vector.tensor_scalar(argbt[:], iotf[:], b2col[:], None, MULT)
                PS2 = lay.tile([128, R], f32, tag="PS2", name="PS2")
                PC2 = lay.tile([128, R], f32, tag="PC2", name="PC2")
                mod_sin_cos(PS2[:], PC2[:], argbt[:], [128, R])

                argl = wsm.tile([128, 2], f32, tag="argl", name="argl")
                nc.vector.tensor_scalar_mul(argl[:, 0:1], b2col[:], float(R))
                nc.vector.tensor_copy(argl[:, 1:2], b2col[:])
                sl = wsm.tile([128, 2], f32, tag="sl", name="sl")
                cl = wsm.tile([128, 2], f32, tag="cl", name="cl")
                mod_sin_cos(sl[:], cl[:], argl[:], [128, 2])

                dd_d = dram.tile([2, TR], f32, tag="dd_d")
                nc.sync.dma_start(
                    bass.AP(tensor=dd_d.tensor, offset=0, ap=[[1, TR], [1, 1]]),
                    dcol[:])
                nc.sync.dma_start(
                    bass.AP(tensor=dd_d.tensor, offset=TR, ap=[[1, TR], [1, 1]]),
                    d512c[:])
                D128all = lay.tile([128, NCH], f32, tag="D128all", name="D128all")
                D512all = wsm.tile([128, NCH], f32, tag="D512all", name="D512all")
                for half in range(2):
                    nc.sync.dma_start(
                        D128all[64 * half:64 * half + 64, :],
                        bass.AP(tensor=dd_d.tensor, offset=half,
                                ap=[[0, 64], [2, NCH]]))
                    nc.sync.dma_start(
                        D512all[64 * half:64 * half + 64, :],
                        bass.AP(tensor=dd_d.tensor, offset=TR + half,
                                ap=[[0, 64], [2, NCH]]))
                L5r = lay.tile([128, NCH], f32, tag="L5r", name="L5r")
                L5i = lay.tile([128, NCH], f32, tag="L5i", name="L5i")
                nc.vector.tensor_scalar(L5r[:], D512all[:], cl[:, 0:1], None, MULT)
                nc.vector.tensor_scalar(L5i[:], D512all[:], sl[:, 0:1], None, MULT)

                # ---------- projections ----------
                wtr_sb = [lay.tile([128, TR], f32r, tag=f"wtr{k}", name=f"wtr{k}")
                          for k in range(4)]
                wc_sb = [lay.tile([128, CC], f32r, tag=f"wc{k}", name=f"wc{k}")
                         for k in range(4)]
                for k in range(4):
                    nc.sync.dma_start(wtr_sb[k][:], WtrT[l, k * 128:(k + 1) * 128, :])
                    nc.sync.dma_start(wc_sb[k][:], WcT[l, k * 128:(k + 1) * 128, :])
                trP = psum2.tile([TR, R], f32, tag="projP", name="trP")
                for k in range(4):
                    nc.tensor.matmul(trP[:], wtr_sb[k][:], xt_sb[k][:],
                                     start=(k == 0), stop=(k == 3))
                atrT = wt("gA", None, f32r)
                nc.scalar.activation(atrT[:TR], trP[:], ACT.Abs)
                ctP = psum2.tile([CC, R], f32, tag="projP", name="ctP")
                for k in range(4):
                    nc.tensor.matmul(ctP[:], wc_sb[k][:], xt_sb[k][:],
                                     start=(k == 0), stop=(k == 3))
                actT = wt("gB", None, f32r)
                nc.scalar.activation(actT[:TR], ctP[:], ACT.Abs)

                s1P = psum2.tile([1, R], f32, tag="statP", name="s1P")
                nc.tensor.matmul(s1P[:], ones64, atrT[:TR], start=True, stop=True)
                s1 = wt("gE", [1, R])
                nc.scalar.copy(s1[:], s1P[:])
                s2P = psum2.tile([1, R], f32, tag="statP", name="s2P")
                nc.tensor.matmul(s2P[:], ones64, actT[:TR], start=True, stop=True)
                den = wt("gF", [1, R])
                nc.vector.tensor_mul(den[:], s1[:], s2P[:])
                nc.vector.tensor_scalar_add(den[:], den[:], 1e-8)
                rden = wt("gE", [1, R])
                nc.vector.reciprocal(rden[:], den[:])
                rd_d = dram.tile([1, R], f32, tag="rd_d")
                nc.sync.dma_start(rd_d[:], rden[:])
                rden64 = wt("gC")
                pb_load(rden64[:TR], rd_d, 0, TR, R)

                A0 = wt("gD")
                nc.vector.tensor_mul(A0[:TR], atrT[:TR].bitcast(f32), rden64[:TR])
                A0_d = dram.tile([TR, R], f32, tag="A0_d")
                nc.sync.dma_start(A0_d[:], A0[:TR])

                Bc = wt("gE")
                nc.vector.tensor_mul(Bc[:TR], actT[:TR].bitcast(f32), PC2[0:CC, :])
                Bs = wt("gF")
                nc.vector.scalar_tensor_tensor(Bs[:TR], actT[:TR].bitcast(f32), -1.0,
                                               PS2[0:CC, :], MULT, MULT)
                bb_d = dram.tile([2 * CC, R], f32, tag="bb_d")
                nc.sync.dma_start(bb_d[0:CC, :], Bc[:TR])
                nc.sync.dma_start(bb_d[CC:2 * CC, :], Bs[:TR])
                BC2 = lay.tile([128, R], f32, tag="BC2", name="BC2")
                BS2 = lay.tile([128, R], f32, tag="BS2", name="BS2")
                for half in range(2):
                    nc.sync.dma_start(BC2[64 * half:64 * half + 64, :],
                                      bass.AP(tensor=bb_d.tensor, offset=0,
                                              ap=[[R, CC], [1, R]]))
                    nc.sync.dma_start(BS2[64 * half:64 * half + 64, :],
                                      bass.AP(tensor=bb_d.tensor, offset=CC * R,
                                              ap=[[R, CC], [1, R]]))

                Dt1T = wt("gD")
                nc.scalar.activation(Dt1T[:TR], iotf[0:TR, :], ACT.Exp,
                                     bias=negam[:], scale=negam[:])
                Dt_d = dram.tile([TR, R], f32, tag="Dt_d")
                nc.sync.dma_start(Dt_d[:], Dt1T[:TR])

                st0_sb = lay.tile([128, NCH], f32, tag="st0_sb", name="st0_sb")
                nc.sync.dma_start(st0_sb[:], st0m[l])

                # ---------- pass 1: scans ----------
                Ballr = lay.tile([128, NCH], f32, tag="Ballr", name="Ballr")
                Balli = lay.tile([128, NCH], f32, tag="Balli", name="Balli")
                c511 = PC2[:, R - 1:R]
                s511 = PS2[:, R - 1:R]
                for g in range(NCH):
                    A2g = wt("gA")
                    pb_load(A2g[0:64, :], A0_d, 2 * g, 64, R)
                    pb_load(A2g[64:128, :], A0_d, 2 * g + 1, 64, R)
                    d0g = wt("gB")
                    nc.vector.tensor_scalar(d0g[:], M2[:], D128all[:, g:g + 1],
                                            None, MULT)
                    ucg = wt("gC")
                    nc.vector.tensor_mul(ucg[:], A2g[:], BC2[:])
                    usg = wt("gD")
                    nc.vector.tensor_mul(usg[:], A2g[:], BS2[:])
                    qr, qi = q_tiles[g]
                    nc.vector.tensor_tensor_scan(qr[:], d0g[:], ucg[:], 0.0, MULT, ADD)
                    nc.vector.tensor_tensor_scan(qi[:], d0g[:], usg[:], 0.0, MULT, ADD)
                    t1 = wsm.tile([128, 1], f32, tag="bx1", name="bx1")
                    nc.vector.tensor_mul(t1[:], qr[:, R - 1:R], c511)
                    t2 = wsm.tile([128, 1], f32, tag="bx2", name="bx2")
                    nc.vector.tensor_mul(t2[:], qi[:, R - 1:R], s511)
                    nc.vector.tensor_sub(Ballr[:, g:g + 1], t1[:], t2[:])
                    t3 = wsm.tile([128, 1], f32, tag="bx3", name="bx3")
                    nc.vector.tensor_mul(t3[:], qr[:, R - 1:R], s511)
                    t4 = wsm.tile([128, 1], f32, tag="bx4", name="bx4")
                    nc.vector.tensor_mul(t4[:], qi[:, R - 1:R], c511)
                    nc.vector.tensor_add(Balli[:, g:g + 1], t3[:], t4[:])

                # ---------- carry exchange ----------
                flag128 = wsm.tile([1, 128], f32, tag="flag128", name="flag128")
                zr128 = wsm.tile([1, 128], f32, tag="zr128", name="zr128")
                nc.vector.memset(zr128[:], 0.0)
                nc.vector.tensor_scalar(flag128[:], zr128[:], flagv[:], None, ADD)
                cin_d = dram.tile([66, 128], f32, tag="cin_d")
                nc.sync.dma_start(
                    bass.AP(tensor=cin_d.tensor, offset=0,
                            ap=[[1, 128], [128, NCH]]),
                    Ballr[:])
                nc.sync.dma_start(
                    bass.AP(tensor=cin_d.tensor, offset=NCH * 128,
                            ap=[[1, 128], [128, NCH]]),
                    Balli[:])
                nc.sync.dma_start(
                    bass.AP(tensor=cin_d.tensor, offset=64 * 128,
                            ap=[[1, 1], [1, 128]]),
                    flag128[:])
                gout_d = dram.tile([NC_ * 66, 128], f32, tag="gout_d")
                nc.gpsimd.collective_compute(
                    "AllGather", mybir.AluOpType.bypass,
                    replica_groups=[list(range(NC_))],
                    ins=[cin_d.opt()], outs=[gout_d.opt()])

                flagrow = wsm.tile([1, NC_], f32, tag="flagrow", name="flagrow")
                nc.sync.dma_start(
                    flagrow[:],
                    bass.AP(tensor=gout_d.tensor, offset=64 * 128,
                            ap=[[1, 1], [66 * 128, NC_]]))
                ef = wsm.tile([1, NC_], f32, tag="ef", name="ef")
                nc.vector.tensor_mul(ef[:], pm_sb[:], flagrow[:])
                ef_d = dram.tile([1, NC_], f32, tag="ef_d")
                nc.sync.dma_start(ef_d[:], ef[:])
                EF128 = wsm.tile([128, NC_], f32, tag="EF128", name="EF128")
                pb_load(EF128[:], ef_d, 0, 128, NC_)

                Sr = wsm.tile([128, NCH], f32, tag="Sx0", name="Sr_init")
                Si = wsm.tile([128, NCH], f32, tag="Sy0", name="Si_init")
                nc.vector.tensor_copy(Sr[:], st0_sb[:])
                nc.vector.memset(Si[:], 0.0)
                for n in range(NC_):
                    Grn = wsm.tile([128, NCH], f32, tag="Grn", name="Grn")
                    nc.sync.dma_start(
                        Grn[:],
                        bass.AP(tensor=gout_d.tensor, offset=66 * 128 * n,
                                ap=[[1, 128], [128, NCH]]))
                    Gin = wsm.tile([128, NCH], f32, tag="Gin", name="Gin")
                    nc.sync.dma_start(
                        Gin[:],
                        bass.AP(tensor=gout_d.tensor,
                                offset=66 * 128 * n + NCH * 128,
                                ap=[[1, 128], [128, NCH]]))
                    ar = wsm.tile([128, NCH], f32, tag="ar", name="ar")
                    nc.vector.scalar_tensor_tensor(
                        ar[:], L5r[:], EF128[:, n:n + 1],
                        OM128[:, n:n + 1].broadcast_to([128, NCH]), MULT, ADD)
                    ai = wsm.tile([128, NCH], f32, tag="ai", name="ai")
                    nc.vector.tensor_scalar(ai[:], L5i[:], EF128[:, n:n + 1],
                                            None, MULT)
                    v1 = wsm.tile([128, NCH], f32, tag="v1", name="v1")
                    nc.vector.tensor_mul(v1[:], ar[:], Sr[:])
                    v2 = wsm.tile([128, NCH], f32, tag="v2", name="v2")
                    nc.vector.tensor_mul(v2[:], ai[:], Si[:])
                    v3 = wsm.tile([128, NCH], f32, tag="v3", name="v3")
                    nc.vector.tensor_sub(v3[:], v1[:], v2[:])
                    v4 = wsm.tile([128, NCH], f32, tag="v1", name="v4")
                    nc.vector.tensor_mul(v4[:], ai[:], Sr[:])
                    v5 = wsm.tile([128, NCH], f32, tag="v2", name="v5")
                    nc.vector.tensor_mul(v5[:], ar[:], Si[:])
                    v6 = wsm.tile([128, NCH], f32, tag="v4", name="v6")
                    nc.vector.tensor_add(v6[:], v4[:], v5[:])
                    Sr_new = wsm.tile([128, NCH], f32, tag="Sx1", name="Sr_new")
                    nc.vector.scalar_tensor_tensor(
                        Sr_new[:], Grn[:], PM128[:, n:n + 1], v3[:], MULT, ADD)
                    Si_new = wsm.tile([128, NCH], f32, tag="Sy1", name="Si_new")
                    nc.vector.scalar_tensor_tensor(
                        Si_new[:], Gin[:], PM128[:, n:n + 1], v6[:], MULT, ADD)
                    Sr, Si = Sr_new, Si_new
                w1t = wsm.tile([128, NCH], f32, tag="v1", name="w1t")
                nc.vector.tensor_scalar(w1t[:], Sr[:], cl[:, 1:2], None, MULT)
                w2t = wsm.tile([128, NCH], f32, tag="v2", name="w2t")
                nc.vector.tensor_scalar(w2t[:], Si[:], sl[:, 1:2], None, MULT)
                QIR = lay.tile([128, NCH], f32, tag="QIR", name="QIR")
                nc.vector.tensor_sub(QIR[:], w1t[:], w2t[:])
                w3t = wsm.tile([128, NCH], f32, tag="v3", name="w3t")
                nc.vector.tensor_scalar(w3t[:], Sr[:], sl[:, 1:2], None, MULT)
                w4t = wsm.tile([128, NCH], f32, tag="v4", name="w4t")
                nc.vector.tensor_scalar(w4t[:], Si[:], cl[:, 1:2], None, MULT)
                QII = lay.tile([128, NCH], f32, tag="QII", name="QII")
                nc.vector.tensor_add(QII[:], w3t[:], w4t[:])

                # ---------- pass 2: correction + features + W0 ----------
                y0P = [psum.tile([128, R], f32, tag=f"y0P{dt}", name=f"y0P{dt}")
                       for dt in range(4)]
                for g in range(NCH):
                    qr, qi = q_tiles[g]
                    DT2g = wt("gA")
                    pb_load(DT2g[0:64, :], Dt_d, 2 * g, 64, R)
                    pb_load(DT2g[64:128, :], Dt_d, 2 * g + 1, 64, R)
                    PDg = wt("gB")
                    nc.vector.tensor_mul(PDg[:], P2[:], DT2g[:])
                    nc.vector.scalar_tensor_tensor(qr[:], PDg[:], QIR[:, g:g + 1],
                                                   qr[:], MULT, ADD)
                    nc.vector.scalar_tensor_tensor(qi[:], PDg[:], QII[:, g:g + 1],
                                                   qi[:], MULT, ADD)
                    sqr = wt("gC")
                    nc.scalar.activation(sqr[:], qr[:], ACT.Square)
                    sqi = wt("gD")
                    nc.scalar.activation(sqi[:], qi[:], ACT.Square)
                    nsq = wt("gE")
                    nc.gpsimd.tensor_add(nsq[:], sqr[:], sqi[:])
                    nn_ = wt("gF")
                    nc.scalar.activation(nn_[:], nsq[:], ACT.Sqrt,
                                         bias=c_sqrt_eps[:])
                    mag = wt("gA")
                    nc.scalar.activation(mag[:], nn_[:], ACT.Ln, bias=1.0)
                    lm = wt("gB")
                    nc.scalar.activation(lm[:], mag[:], ACT.Ln)
                    ld = wt("gC")
                    nc.scalar.activation(ld[:], nn_[:], ACT.Ln)
                    df = wt("gD")
                    nc.gpsimd.tensor_sub(df[:], lm[:], ld[:])
                    Gt = wt("gE")
                    nc.scalar.activation(Gt[:], df[:], ACT.Exp)
                    gc = wt("gF")
                    nc.vector.tensor_mul(gc[:], Gt[:], PC2[:])
                    gs = wt("gA")
                    nc.vector.tensor_mul(gs[:], Gt[:], PS2[:])
                    t1 = wt("gB")
                    nc.vector.tensor_mul(t1[:], gs[:], qr[:])
                    t2 = wt("gC")
                    nc.vector.tensor_mul(t2[:], gc[:], qi[:])
                    fsin = wt("gD", None, f32r)
                    nc.vector.tensor_add(fsin[:], t1[:], t2[:])
                    t3 = wt("gB")
                    nc.vector.tensor_mul(t3[:], gc[:], qr[:])
                    t4 = wt("gC")
                    nc.vector.tensor_mul(t4[:], gs[:], qi[:])
                    fcos = wt("gE", None, f32r)
                    nc.vector.tensor_sub(fcos[:], t3[:], t4[:])
                    w0s = wstream.tile([128, D], f32r, tag="w0s", name="w0s")
                    nc.sync.dma_start(w0s[:], W0T[l, g * 128:(g + 1) * 128, :])
                    w0c = wstream.tile([128, D], f32r, tag="w0c", name="w0c")
                    nc.sync.dma_start(w0c[:], W0T[l, TR * CC + g * 128:
                                                 TR * CC + (g + 1) * 128, :])
                    for dt in range(4):
                        nc.tensor.matmul(y0P[dt][:], w0s[:, dt * 128:(dt + 1) * 128],
                                         fsin[:], start=(g == 0), stop=False)
                        nc.tensor.matmul(y0P[dt][:], w0c[:, dt * 128:(dt + 1) * 128],
                                         fcos[:],
                                         start=False, stop=(g == NCH - 1))

                # ---------- LN0 + W1 + LN1 ----------
                lnc = lay.tile([128, 24], f32, tag="lnc", name="lnc")
                nc.sync.dma_start(lnc[:], lnp[l])
                z0 = layer_norm(y0P, lnc, 16, 0, 4, "z0")
                w1sb = [wstream.tile([128, D], f32r,
                                        tag=("w0s" if k % 2 == 0 else "w0c"),
                                        name=f"w1sb{k}") for k in range(4)]
                for k in range(4):
                    nc.sync.dma_start(w1sb[k][:], W1T[l, k * 128:(k + 1) * 128, :])
                y1P = [psum.tile([128, R], f32, tag=f"y0P{dt}", name=f"y1P{dt}")
                       for dt in range(4)]
                for dt in range(4):
                    for k in range(4):
                        nc.tensor.matmul(y1P[dt][:],
                                         w1sb[k][:, dt * 128:(dt + 1) * 128],
                                         z0[k][:], start=(k == 0), stop=(k == 3))
                z1 = layer_norm(y1P, lnc, 20, 8, 12, "z1")

                if l < L - 1:
                    for k in range(4):
                        nc.vector.tensor_add(xt_sb[k][:], xt_sb[k][:].bitcast(f32),
                                             z1[k][:].bitcast(f32))
                else:
                    for dt in range(4):
                        nc.sync.dma_start(zT[dt * 128:(dt + 1) * 128, :],
                                          z1[dt][:].bitcast(f32))

    _split_multiwaits(nc)
    return nc


def kernel(**inputs):
    from concourse.bass_utils import run_bass_kernel_spmd

    x = np.ascontiguousarray(np.asarray(inputs["x"], np.float32))
    state = np.asarray(inputs["state"], np.float32)
    start = np.asarray(inputs["start"])
    Wtr = np.asarray(inputs["Wtr"], np.float32)
    Wc = np.asarray(inputs["Wc"], np.float32)
    a = np.asarray(inputs["a"], np.float32)
    b = np.asarray(inputs["b"], np.float32)
    W0 = np.asarray(inputs["W0"], np.float32)
    b0 = np.asarray(inputs["b0"], np.float32)
    g0 = np.asarray(inputs["g0"], np.float32)
    beta0 = np.asarray(inputs["beta0"], np.float32)
    W1 = np.asarray(inputs["W1"], np.float32)
    b1 = np.asarray(inputs["b1"], np.float32)
    g1 = np.asarray(inputs["g1"], np.float32)
    beta1 = np.asarray(inputs["beta1"], np.float32)

    if "nc" not in _CACHE:
        _CACHE["nc"] = _build()
    nc = _CACHE["nc"]

    WtrT = np.ascontiguousarray(Wtr.transpose(0, 2, 1))
    WcT = np.ascontiguousarray(Wc.transpose(0, 2, 1))
    W0T = np.ascontiguousarray(W0.transpose(0, 2, 1))
    W1T = np.ascontiguousarray(W1.transpose(0, 2, 1))
    avec = np.ascontiguousarray(a.reshape(L, TR, 1))
    bvec = np.ascontiguousarray(b.reshape(L, CC, 1))
    st0m = np.ascontiguousarray(
        state.reshape(L, TR * CC).reshape(L, NCH, 128).transpose(0, 2, 1))
    lnp = np.zeros((L, 128, 24), np.float32)
    for li in range(L):
        for idx, vec in enumerate([g0[li], beta0[li], g1[li], beta1[li],
                                   b0[li], b1[li]]):
            lnp[li, :, idx * 4:(idx + 1) * 4] = vec.reshape(4, 128).T
    startf_full = start.astype(np.float32)

    shared = dict(WtrT=WtrT, WcT=WcT, W0T=W0T, W1T=W1T, avec=avec, bvec=bvec,
                  st0m=st0m, lnp=lnp)
    in_maps = []
    for c in range(NC_):
        im = dict(shared)
        im["xT"] = np.ascontiguousarray(x[c * R:(c + 1) * R].T)
        im["startf"] = np.ascontiguousarray(startf_full[c * R:(c + 1) * R]
                                            .reshape(1, R))
        pm = np.zeros((1, NC_), np.float32)
        pm[0, :c] = 1.0
        im["pmask"] = pm
        in_maps.append(im)

    res = run_bass_kernel_spmd(nc, in_maps, core_ids=list(range(NC_)),
                               **_CACHE.get("run_kwargs", {}))
    _CACHE["last_result"] = res
    out = np.empty((T, D), np.float32)
    for c in range(NC_):
        out[c * R:(c + 1) * R] = res.results[c]["zT"].T
    return out
